# revision 15
# baseline (speedup 1.0000x reference)
"""Trainium2 Bass kernel for nn_CASCADES_v8_ResonantCore (moe_routing):

Computation (per batch b):
    centroid = 0.7*x[b,-1,:] + 0.3*mean_s(x[b])
    w = softmax(cos_sim(centroid, core_keys)/TEMP)      # [K]
    Lam = sum_k w[k] * core_pool[k]                     # [R,R]
    out[b] = ((x[b] @ V^T) @ Lam^T) @ U^T               # [S,D]

Strategy (8 cores, data-parallel over (batch, seq-half)):
  - Host: exact f64 routing; W_b = (U @ Lam_b)^T folded to one [R, D]
    weight per batch.  Output is written int8 with a per-column scale
    s_d = 8*sigma_d/127 (sigma_d^2 = W_d^T (V V^T) W_d) folded into
    the weight; host dequantizes.  f32->int8 on DVE/ACT rounds-to-
    nearest and saturates (HW-probed), so |err| <= s_d/2 ~ 0.03 sigma.
  - Reads: 512 KiB sub-DMAs on the sync ring (consts first), so the
    first matmul starts ~11.5 us instead of ~18.6 us.  Writes: eager
    per-strip int8 on the scalar ring.
  - Device per seq-group: V replicated 16x along the free dim in SBUF,
    so 32 accumulating matmuls produce the replicated xv^T [128, n]
    directly in PSUM; one copy to SBUF; then 8 expansion matmuls per
    128-row strip with [128,512] f32->int8 drains alternating DVE/ACT.
  - Groups are (512,512,512,256,256) rows: the smaller tail groups
    shrink the serial mm1->mm2->drain->write chain after the 16.8 MiB
    read stream ends (~358 GB/s sustained per core).
  - HBM traffic per core: 16.8 MiB read + 8.4 MiB write ~= 25.3 MiB.
"""

import sys

sys.path.insert(0, "/opt/trn_rl_repo")

import contextlib

import ml_dtypes
import numpy as np

import concourse.bass as bass  # noqa: F401  (registers bass types)
import concourse.tile as tile
from concourse import bacc, mybir
from concourse.bass_utils import run_bass_kernel_spmd

BF16 = ml_dtypes.bfloat16

B, S, D, R, K = 4, 4096, 4096, 8, 4
NCORES = 8
SH = S // 2     # 2048 seq rows per core
G = 4           # 512-row read-tile groups per core
SG = SH // G    # 512
NCH = D // 128  # 32 d-chunks
TPG = 2         # x tiles per 512-row group ([128, 8192] each)
CPT = NCH // TPG  # 16 d-chunks per x tile
NSUB = 4        # read sub-DMAs per tile (512 KiB each)
CPS = CPT // NSUB
EPS, TEMP = 1e-8, 0.05
QC = 8.0        # int8 scale: s_d = QC * sigma_d / 127

_cache = {}


def build_fused():
    """xtp [1024, 8192] bf16, vt [128, 256] bf16, wt [8, 4096] bf16
    (per-column-scaled W'), rp [8, 128] bf16 -> out [128, 65536] int8
    with out[p, q*4096 + d] = out_rows[q*128 + p, d], q = row-block."""
    rep = 128 // R
    nc = bacc.Bacc("TRN2", target_bir_lowering=False, debug=False)
    xtp = nc.dram_tensor(
        "xtp", [G * TPG * 128, CPT * SG], mybir.dt.bfloat16, kind="ExternalInput"
    ).ap()
    vt = nc.dram_tensor("vt", [128, NCH * R], mybir.dt.bfloat16, kind="ExternalInput").ap()
    wt = nc.dram_tensor("wt", [R, D], mybir.dt.bfloat16, kind="ExternalInput").ap()
    rp = nc.dram_tensor("rp", [R, 128], mybir.dt.bfloat16, kind="ExternalInput").ap()
    out = nc.dram_tensor("out", [128, (SH // 128) * D], mybir.dt.int8,
                         kind="ExternalOutput").ap()

    with tile.TileContext(nc) as tc:
        with contextlib.ExitStack() as ctx:
            cpool = ctx.enter_context(tc.tile_pool(name="consts", bufs=1))
            xpool = ctx.enter_context(tc.tile_pool(name="x", bufs=6))
            vrpool = ctx.enter_context(tc.tile_pool(name="xvr", bufs=2))
            opool = ctx.enter_context(tc.tile_pool(name="ob", bufs=8))
            psA = ctx.enter_context(tc.tile_pool(name="psA", bufs=2, space="PSUM"))
            psB = ctx.enter_context(tc.tile_pool(name="psB", bufs=6, space="PSUM"))

            # consts head the ring
            vt_sb = cpool.tile([128, NCH * R], mybir.dt.bfloat16)
            nc.sync.dma_start(vt_sb[:], vt[:])
            wt_sb = cpool.tile([R, D], mybir.dt.bfloat16)
            nc.sync.dma_start(wt_sb[:], wt[:])
            rp_sb = cpool.tile([R, 128], mybir.dt.bfloat16)
            nc.sync.dma_start(rp_sb[:], rp[:])

            # the whole x stream as 512 KiB sub-DMAs
            xs = []
            for t in range(G * TPG):
                xt = xpool.tile([128, CPT * SG], mybir.dt.bfloat16, tag="xs")
                for q in range(NSUB):
                    cols = slice(q * CPS * SG, (q + 1) * CPS * SG)
                    nc.sync.dma_start(xt[:, cols], xtp[t * 128:(t + 1) * 128, cols])
                xs.append(xt)

            # wtr = wt[p%8]/16 via 8 repmat matmuls (also HAM warmup)
            wtr_sb = cpool.tile([128, D], mybir.dt.bfloat16)
            for j in range(D // 512):
                psw = psB.tile([128, 512], mybir.dt.float32, tag="ps")
                nc.tensor.matmul(psw[:], rp_sb[:], wt_sb[:, j * 512:(j + 1) * 512],
                                 start=True, stop=True)
                if j % 2 == 0:
                    nc.vector.tensor_copy(wtr_sb[:, j * 512:(j + 1) * 512], psw[:])
                else:
                    nc.scalar.copy(wtr_sb[:, j * 512:(j + 1) * 512], psw[:])
            # vtr: V replicated 16x along the free dim, 16 strided copies
            vtr_sb = cpool.tile([128, NCH * 128], mybir.dt.bfloat16)
            vtr_v = vtr_sb[:].rearrange("p (c t j) -> p c t j", t=rep, j=R)
            vt_v = vt_sb[:].rearrange("p (c j) -> p c j", j=R)
            for t in range(rep):
                if t % 2 == 0:
                    nc.vector.tensor_copy(vtr_v[:, :, t, :], vt_v)
                else:
                    nc.scalar.copy(vtr_v[:, :, t, :], vt_v)

            GROUPS = [(0, 512), (512, 512), (1024, 512), (1536, 256), (1792, 256)]
            NG = len(GROUPS)
            xvr_of = {}
            ob_cur = [None]

            def mm1_chunk(k, ps_xv, ch):
                rs, n = GROUPS[k]
                sl = rs % SG
                t2, c = divmod(ch, CPT)
                xt = xs[(rs // SG) * TPG + t2]
                nc.tensor.matmul(
                    ps_xv[:],
                    vtr_sb[:, ch * 128:(ch + 1) * 128],
                    xt[:, c * SG + sl:c * SG + sl + n],
                    start=(ch == 0),
                    stop=(ch == NCH - 1),
                )

            def xvr_chain(k, ps_xv):
                rs, n = GROUPS[k]
                xvr = vrpool.tile([128, n], mybir.dt.bfloat16, tag="xvr", name="xvr")
                if k % 2 == 0:
                    nc.vector.tensor_copy(xvr[:], ps_xv[:])
                else:
                    nc.scalar.copy(xvr[:], ps_xv[:])
                xvr_of[k] = xvr

            def mm2_unit(k, u):
                # one [128,512] column block of out strip u//8
                rs, n = GROUPS[k]
                i, j = divmod(u, D // 512)
                if j == 0:
                    ob_cur[0] = opool.tile([128, D], mybir.dt.int8, tag="ob",
                                           name="ob")
                ob = ob_cur[0]
                ps = psB.tile([128, 512], mybir.dt.float32, tag="ps", name="ps")
                nc.tensor.matmul(
                    ps[:],
                    xvr_of[k][:, i * 128:(i + 1) * 128],
                    wtr_sb[:, j * 512:(j + 1) * 512],
                    start=True, stop=True,
                )
                dst = ob[:, j * 512:(j + 1) * 512]
                if u % 2 == 0:
                    nc.vector.tensor_copy(dst, ps[:])
                else:
                    nc.scalar.copy(dst, ps[:])
                if j == D // 512 - 1:
                    q = rs // 128 + i
                    nc.scalar.dma_start(out[:, q * D:(q + 1) * D], ob[:])

            # group 0: plain mm1 (wtr/vtr build above covers PE warmup)
            ps_prev = psA.tile([128, 512], mybir.dt.float32, tag="psxv", name="psxv")
            for ch in range(NCH):
                mm1_chunk(0, ps_prev, ch)
            xvr_chain(0, ps_prev)
            # groups k>=1: weave mm1(k) chunks 4:4 with mm2(k-1) units
            for k in range(1, NG):
                rs, n = GROUPS[k]
                n_prev = GROUPS[k - 1][1]
                nu = (n_prev // 128) * (D // 512)   # 32 or 16 units
                ps_xv = psA.tile([128, n], mybir.dt.float32, tag="psxv", name="psxv")
                u = 0
                for m in range(8):
                    for ch in range(4 * m, 4 * m + 4):
                        mm1_chunk(k, ps_xv, ch)
                    take = nu // 8 + (1 if m < nu % 8 else 0)
                    for _ in range(take):
                        mm2_unit(k - 1, u)
                        u += 1
                while u < nu:
                    mm2_unit(k - 1, u)
                    u += 1
                xvr_chain(k, ps_xv)
                ps_prev = ps_xv
            # trailing: last group's own mm2
            for u in range((GROUPS[NG - 1][1] // 128) * (D // 512)):
                mm2_unit(NG - 1, u)

    nc.compile()
    return nc


def _get_kernels():
    if "k" not in _cache:
        _cache["k"] = build_fused()
    return _cache["k"]


def _vt_layout(V, d, r):
    """[128, (d//128)*r] bf16 with vt[p, c*r + j] = V[j, c*128 + p]."""
    nch = d // 128
    return np.ascontiguousarray(
        V.reshape(r, nch, 128).transpose(2, 1, 0).reshape(128, nch * r)
    ).astype(BF16)


def _routing_weights(x, V_shared, U_shared, core_pool, core_keys):
    """Exact f64 routing on host -> per-batch (W'_b [R, D] bf16 scaled by
    1/s_d, s [B, D] f32 dequant scales)."""
    mean = x.mean(axis=1, dtype=np.float64)  # [B, D]
    centroid = 0.7 * x[:, -1, :].astype(np.float64) + 0.3 * mean
    c_n = centroid / np.maximum(
        np.linalg.norm(centroid, axis=-1, keepdims=True), EPS
    )
    kk = core_keys.astype(np.float64)
    k_n = kk / np.maximum(np.linalg.norm(kk, axis=-1, keepdims=True), EPS)
    sim = c_n @ k_n.T  # [B, K]
    logits = sim / TEMP
    e = np.exp(logits - logits.max(axis=-1, keepdims=True))
    w = e / e.sum(axis=-1, keepdims=True)
    Lam = np.einsum("bk,kij->bij", w, core_pool.astype(np.float64))  # [B, R, R]
    Wb = np.einsum("dr,brj->bjd", U_shared.astype(np.float64), Lam)  # [B, R, D]
    Vf = V_shared.astype(np.float64)
    C = Vf @ Vf.T  # [R, R]
    sig = np.sqrt(np.einsum("bjd,jk,bkd->bd", Wb, C, Wb))  # [B, D]
    s = (QC / 127.0) * np.maximum(sig, 1e-12)  # [B, D]
    wt_b = [np.ascontiguousarray(Wb[b] / s[b][None, :]).astype(BF16) for b in range(B)]
    return wt_b, s.astype(np.float32)


def _pack_xtp(xshard):
    """[SH, D] f32 -> [1024, 8192] bf16: tile t=g*2+t2 row p col c*SG+s
    = x[g*512 + s, (t2*16 + c)*128 + p]."""
    v = np.ascontiguousarray(
        xshard.reshape(G, SG, TPG, CPT, 128).transpose(0, 2, 4, 3, 1)
    )
    return v.reshape(G * TPG * 128, CPT * SG).astype(BF16)


def _rp_layout(r):
    """[r, 128] bf16, rp[k, m] = (m % r == k)/16: partition replicator."""
    m = np.arange(128)
    return ((m[None, :] % r == np.arange(r)[:, None]) / 16.0).astype(BF16)


def _shard_inputs(x, V_shared, U_shared, core_pool, core_keys):
    vt_np = _vt_layout(V_shared.astype(np.float32), D, R)
    rp_np = _rp_layout(R)
    wt_b, s = _routing_weights(x, V_shared, U_shared, core_pool, core_keys)
    in_maps = []
    for c in range(NCORES):
        b, h = c // 2, c % 2
        xtp_c = _pack_xtp(x[b, h * SH:(h + 1) * SH, :])
        in_maps.append({"xtp": xtp_c, "vt": vt_np, "wt": wt_b[b], "rp": rp_np})
    return in_maps, s


def kernel(x, V_shared, U_shared, core_pool, core_keys):
    x = np.asarray(x)
    V_shared = np.asarray(V_shared)
    U_shared = np.asarray(U_shared)
    core_pool = np.asarray(core_pool)
    core_keys = np.asarray(core_keys)

    nc = _get_kernels()
    core_ids = list(range(NCORES))
    in_maps, s = _shard_inputs(x, V_shared, U_shared, core_pool, core_keys)
    res = run_bass_kernel_spmd(nc, in_maps, core_ids).results

    out = np.empty((B, S, D), dtype=np.float32)
    for c in core_ids:
        b, h = c // 2, c % 2
        a = res[c]["out"].reshape(128, SH // 128, D).transpose(1, 0, 2)
        out[b, h * SH:(h + 1) * SH, :] = (
            a.reshape(SH, D).astype(np.float32) * s[b][None, :]
        )
    return out


# revision 17
# speedup vs baseline: 1.0895x; 1.0895x over previous
"""Trainium2 Bass kernel for nn_CASCADES_v8_ResonantCore (moe_routing):

Computation (per batch b):
    centroid = 0.7*x[b,-1,:] + 0.3*mean_s(x[b])
    w = softmax(cos_sim(centroid, core_keys)/TEMP)      # [K]
    Lam = sum_k w[k] * core_pool[k]                     # [R,R]
    out[b] = ((x[b] @ V^T) @ Lam^T) @ U^T               # [S,D]

Strategy (8 cores, data-parallel over (batch, seq-half)):
  - Host: exact f64 routing; W_b = (U @ Lam_b)^T folded to one [R, D]
    weight per batch.  Output is written int8 with a per-column scale
    s_d = 8*sigma_d/127 (sigma_d^2 = W_d^T (V V^T) W_d) folded into
    the weight; host dequantizes.  f32->int8 on DVE/ACT rounds-to-
    nearest and saturates (HW-probed), so |err| <= s_d/2 ~ 0.03 sigma.
  - Reads: 512 KiB sub-DMAs on the sync ring (consts first), so the
    first matmul starts ~11.5 us instead of ~18.6 us.  Writes: eager
    per-strip int8 on the scalar ring.
  - Device per seq-group: V replicated 16x along the free dim in SBUF,
    so 32 accumulating matmuls produce the replicated xv^T [128, n]
    directly in PSUM; one copy to SBUF; then 8 expansion matmuls per
    128-row strip with [128,512] f32->int8 drains alternating DVE/ACT.
  - Groups are (512,512,512,256,256) rows: the smaller tail groups
    shrink the serial mm1->mm2->drain->write chain after the 16.8 MiB
    read stream ends (~358 GB/s sustained per core).
  - HBM traffic per core: 16.8 MiB read + 8.4 MiB write ~= 25.3 MiB.
"""

import sys

sys.path.insert(0, "/opt/trn_rl_repo")

import contextlib

import ml_dtypes
import numpy as np

import concourse.bass as bass  # noqa: F401  (registers bass types)
import concourse.tile as tile
from concourse import bacc, mybir
from concourse.bass_utils import run_bass_kernel_spmd

BF16 = ml_dtypes.bfloat16

B, S, D, R, K = 4, 4096, 4096, 8, 4
NCORES = 8
SH = S // 2     # 2048 seq rows per core
G = 4           # 512-row read-tile groups per core
SG = SH // G    # 512
NCH = D // 128  # 32 d-chunks
TPG = 2         # x tiles per 512-row group ([128, 8192] each)
CPT = NCH // TPG  # 16 d-chunks per x tile
NSUB = 4        # read sub-DMAs per tile (512 KiB each)
CPS = CPT // NSUB
EPS, TEMP = 1e-8, 0.05
QC = 8.0        # int8 scale: s_d = QC * sigma_d / 127

_cache = {}


def build_fused():
    """xtp [1024, 8192] bf16, vt [128, 256] bf16, wt [8, 4096] bf16
    (per-column-scaled W'), rp [8, 128] bf16 -> out [128, 65536] int8
    with out[p, q*4096 + d] = out_rows[q*128 + p, d], q = row-block."""
    rep = 128 // R
    nc = bacc.Bacc("TRN2", target_bir_lowering=False, debug=False)
    xtp = nc.dram_tensor(
        "xtp", [G * TPG * 128, CPT * SG], mybir.dt.bfloat16, kind="ExternalInput"
    ).ap()
    vt = nc.dram_tensor("vt", [128, NCH * R], mybir.dt.bfloat16, kind="ExternalInput").ap()
    wt = nc.dram_tensor("wt", [R, D], mybir.dt.bfloat16, kind="ExternalInput").ap()
    rp = nc.dram_tensor("rp", [R, 128], mybir.dt.bfloat16, kind="ExternalInput").ap()
    out = nc.dram_tensor("out", [128, (SH // 128) * D], mybir.dt.int8,
                         kind="ExternalOutput").ap()

    with tile.TileContext(nc) as tc:
        with contextlib.ExitStack() as ctx:
            cpool = ctx.enter_context(tc.tile_pool(name="consts", bufs=1))
            xpool = ctx.enter_context(tc.tile_pool(name="x", bufs=6))
            vrpool = ctx.enter_context(tc.tile_pool(name="xvr", bufs=2))
            opool = ctx.enter_context(tc.tile_pool(name="ob", bufs=8))
            psA = ctx.enter_context(tc.tile_pool(name="psA", bufs=2, space="PSUM"))
            psB = ctx.enter_context(tc.tile_pool(name="psB", bufs=3, space="PSUM"))

            # consts head the ring
            vt_sb = cpool.tile([128, NCH * R], mybir.dt.bfloat16)
            nc.sync.dma_start(vt_sb[:], vt[:])
            wt_sb = cpool.tile([R, D], mybir.dt.bfloat16)
            nc.sync.dma_start(wt_sb[:], wt[:])
            rp_sb = cpool.tile([R, 128], mybir.dt.bfloat16)
            nc.sync.dma_start(rp_sb[:], rp[:])

            # the whole x stream as 512 KiB sub-DMAs
            xs = []
            for t in range(G * TPG):
                xt = xpool.tile([128, CPT * SG], mybir.dt.bfloat16, tag="xs")
                for q in range(NSUB):
                    cols = slice(q * CPS * SG, (q + 1) * CPS * SG)
                    nc.sync.dma_start(xt[:, cols], xtp[t * 128:(t + 1) * 128, cols])
                xs.append(xt)

            # wtr = wt[p%8]/16 via 8 repmat matmuls (also HAM warmup)
            wtr_sb = cpool.tile([128, D], mybir.dt.bfloat16)
            for u in range(D // 1024):
                psw = psB.tile([128, 1024], mybir.dt.float32, tag="ps", name="psw")
                for h in range(2):
                    j = u * 2 + h
                    nc.tensor.matmul(psw[:, h * 512:(h + 1) * 512], rp_sb[:],
                                     wt_sb[:, j * 512:(j + 1) * 512],
                                     start=True, stop=True)
                if u % 2 == 0:
                    nc.vector.tensor_copy(wtr_sb[:, u * 1024:(u + 1) * 1024], psw[:])
                else:
                    nc.scalar.copy(wtr_sb[:, u * 1024:(u + 1) * 1024], psw[:])
            # vtr: V replicated 16x along the free dim, 16 strided copies
            vtr_sb = cpool.tile([128, NCH * 128], mybir.dt.bfloat16)
            vtr_v = vtr_sb[:].rearrange("p (c t j) -> p c t j", t=rep, j=R)
            vt_v = vt_sb[:].rearrange("p (c j) -> p c j", j=R)
            for t in range(rep):
                if t % 2 == 0:
                    nc.vector.tensor_copy(vtr_v[:, :, t, :], vt_v)
                else:
                    nc.scalar.copy(vtr_v[:, :, t, :], vt_v)

            GROUPS = [(0, 512), (512, 512), (1024, 512), (1536, 256), (1792, 256)]
            for k, (rs, n) in enumerate(GROUPS):
                # ---- mm1: replicated xv^T [128, n] over 32 d-chunks ----
                ps_xv = psA.tile([128, n], mybir.dt.float32, tag="psxv", name="psxv")
                sl = rs % SG
                for ch in range(NCH):
                    t2, c = divmod(ch, CPT)
                    xt = xs[(rs // SG) * TPG + t2]
                    nc.tensor.matmul(
                        ps_xv[:],
                        vtr_sb[:, ch * 128:(ch + 1) * 128],
                        xt[:, c * SG + sl:c * SG + sl + n],
                        start=(ch == 0),
                        stop=(ch == NCH - 1),
                    )
                xvr = vrpool.tile([128, n], mybir.dt.bfloat16, tag="xvr", name="xvr")
                nc.vector.tensor_copy(xvr[:, :n // 2], ps_xv[:, :n // 2])
                nc.scalar.copy(xvr[:, n // 2:], ps_xv[:, n // 2:])

                # ---- mm2: out strips [128, 4096] int8 = xv @ W'^T ----
                for i in range(n // 128):
                    ob = opool.tile([128, D], mybir.dt.int8, tag="ob", name="ob")
                    for u in range(D // 1024):
                        ps = psB.tile([128, 1024], mybir.dt.float32, tag="ps", name="ps")
                        for h in range(2):
                            j = u * 2 + h
                            nc.tensor.matmul(
                                ps[:, h * 512:(h + 1) * 512],
                                xvr[:, i * 128:(i + 1) * 128],
                                wtr_sb[:, j * 512:(j + 1) * 512],
                                start=True, stop=True,
                            )
                        dst = ob[:, u * 1024:(u + 1) * 1024]
                        if u % 2 == 0:
                            nc.vector.tensor_copy(dst, ps[:])
                        else:
                            nc.scalar.copy(dst, ps[:])
                    q = rs // 128 + i
                    nc.scalar.dma_start(out[:, q * D:(q + 1) * D], ob[:])

    nc.compile()
    return nc


def _get_kernels():
    if "k" not in _cache:
        _cache["k"] = build_fused()
    return _cache["k"]


def _vt_layout(V, d, r):
    """[128, (d//128)*r] bf16 with vt[p, c*r + j] = V[j, c*128 + p]."""
    nch = d // 128
    return np.ascontiguousarray(
        V.reshape(r, nch, 128).transpose(2, 1, 0).reshape(128, nch * r)
    ).astype(BF16)


def _routing_weights(x, V_shared, U_shared, core_pool, core_keys):
    """Exact f64 routing on host -> per-batch (W'_b [R, D] bf16 scaled by
    1/s_d, s [B, D] f32 dequant scales)."""
    mean = x.mean(axis=1, dtype=np.float64)  # [B, D]
    centroid = 0.7 * x[:, -1, :].astype(np.float64) + 0.3 * mean
    c_n = centroid / np.maximum(
        np.linalg.norm(centroid, axis=-1, keepdims=True), EPS
    )
    kk = core_keys.astype(np.float64)
    k_n = kk / np.maximum(np.linalg.norm(kk, axis=-1, keepdims=True), EPS)
    sim = c_n @ k_n.T  # [B, K]
    logits = sim / TEMP
    e = np.exp(logits - logits.max(axis=-1, keepdims=True))
    w = e / e.sum(axis=-1, keepdims=True)
    Lam = np.einsum("bk,kij->bij", w, core_pool.astype(np.float64))  # [B, R, R]
    Wb = np.einsum("dr,brj->bjd", U_shared.astype(np.float64), Lam)  # [B, R, D]
    Vf = V_shared.astype(np.float64)
    C = Vf @ Vf.T  # [R, R]
    sig = np.sqrt(np.einsum("bjd,jk,bkd->bd", Wb, C, Wb))  # [B, D]
    s = (QC / 127.0) * np.maximum(sig, 1e-12)  # [B, D]
    wt_b = [np.ascontiguousarray(Wb[b] / s[b][None, :]).astype(BF16) for b in range(B)]
    return wt_b, s.astype(np.float32)


def _pack_xtp(xshard):
    """[SH, D] f32 -> [1024, 8192] bf16: tile t=g*2+t2 row p col c*SG+s
    = x[g*512 + s, (t2*16 + c)*128 + p]."""
    v = np.ascontiguousarray(
        xshard.reshape(G, SG, TPG, CPT, 128).transpose(0, 2, 4, 3, 1)
    )
    return v.reshape(G * TPG * 128, CPT * SG).astype(BF16)


def _rp_layout(r):
    """[r, 128] bf16, rp[k, m] = (m % r == k)/16: partition replicator."""
    m = np.arange(128)
    return ((m[None, :] % r == np.arange(r)[:, None]) / 16.0).astype(BF16)


def _shard_inputs(x, V_shared, U_shared, core_pool, core_keys):
    vt_np = _vt_layout(V_shared.astype(np.float32), D, R)
    rp_np = _rp_layout(R)
    wt_b, s = _routing_weights(x, V_shared, U_shared, core_pool, core_keys)
    in_maps = []
    for c in range(NCORES):
        b, h = c // 2, c % 2
        xtp_c = _pack_xtp(x[b, h * SH:(h + 1) * SH, :])
        in_maps.append({"xtp": xtp_c, "vt": vt_np, "wt": wt_b[b], "rp": rp_np})
    return in_maps, s


def kernel(x, V_shared, U_shared, core_pool, core_keys):
    x = np.asarray(x)
    V_shared = np.asarray(V_shared)
    U_shared = np.asarray(U_shared)
    core_pool = np.asarray(core_pool)
    core_keys = np.asarray(core_keys)

    nc = _get_kernels()
    core_ids = list(range(NCORES))
    in_maps, s = _shard_inputs(x, V_shared, U_shared, core_pool, core_keys)
    res = run_bass_kernel_spmd(nc, in_maps, core_ids).results

    out = np.empty((B, S, D), dtype=np.float32)
    for c in core_ids:
        b, h = c // 2, c % 2
        a = res[c]["out"].reshape(128, SH // 128, D).transpose(1, 0, 2)
        out[b, h * SH:(h + 1) * SH, :] = (
            a.reshape(SH, D).astype(np.float32) * s[b][None, :]
        )
    return out


# revision 18
# speedup vs baseline: 1.2391x; 1.1373x over previous
"""Trainium2 Bass kernel for nn_CASCADES_v8_ResonantCore (moe_routing):

Computation (per batch b):
    centroid = 0.7*x[b,-1,:] + 0.3*mean_s(x[b])
    w = softmax(cos_sim(centroid, core_keys)/TEMP)      # [K]
    Lam = sum_k w[k] * core_pool[k]                     # [R,R]
    out[b] = ((x[b] @ V^T) @ Lam^T) @ U^T               # [S,D]

Strategy (8 cores, data-parallel over (batch, seq-half)):
  - Host: exact f64 routing; W_b = (U @ Lam_b)^T folded to one [R, D]
    weight per batch.  Output is written int8 with a per-column scale
    s_d = 8*sigma_d/127 (sigma_d^2 = W_d^T (V V^T) W_d) folded into
    the weight; host dequantizes.  f32->int8 on DVE/ACT rounds-to-
    nearest and saturates (HW-probed), so |err| <= s_d/2 ~ 0.03 sigma.
  - Reads: 512 KiB sub-DMAs on the sync ring (consts first), so the
    first matmul starts ~11.5 us instead of ~18.6 us.  Writes: eager
    per-strip int8 on the scalar ring.
  - Device per seq-group: V replicated 16x along the free dim in SBUF,
    so 32 accumulating matmuls produce the replicated xv^T [128, n]
    directly in PSUM; one copy to SBUF; then 8 expansion matmuls per
    128-row strip with [128,512] f32->int8 drains alternating DVE/ACT.
  - Groups are (512,512,512,256,256) rows: the smaller tail groups
    shrink the serial mm1->mm2->drain->write chain after the 16.8 MiB
    read stream ends (~358 GB/s sustained per core).
  - HBM traffic per core: 16.8 MiB read + 8.4 MiB write ~= 25.3 MiB.
"""

import sys

sys.path.insert(0, "/opt/trn_rl_repo")

import contextlib

import ml_dtypes
import numpy as np

import concourse.bass as bass  # noqa: F401  (registers bass types)
import concourse.tile as tile
from concourse import bacc, mybir
from concourse.bass_utils import run_bass_kernel_spmd

BF16 = ml_dtypes.bfloat16

B, S, D, R, K = 4, 4096, 4096, 8, 4
NCORES = 8
SH = S // 2     # 2048 seq rows per core
G = 4           # 512-row read-tile groups per core
SG = SH // G    # 512
NCH = D // 128  # 32 d-chunks
TPG = 2         # x tiles per 512-row group ([128, 8192] each)
CPT = NCH // TPG  # 16 d-chunks per x tile
NSUB = 4        # read sub-DMAs per tile (512 KiB each)
CPS = CPT // NSUB
EPS, TEMP = 1e-8, 0.05
QC = 8.0        # int8 scale: s_d = QC * sigma_d / 127

_cache = {}


def build_fused():
    """xtp [1024, 8192] bf16, vt [128, 256] bf16, wt [8, 4096] bf16
    (per-column-scaled W'), rp [8, 128] bf16 -> out [128, 65536] int8
    with out[p, q*4096 + d] = out_rows[q*128 + p, d], q = row-block."""
    rep = 128 // R
    nc = bacc.Bacc("TRN2", target_bir_lowering=False, debug=False)
    xtp = nc.dram_tensor(
        "xtp", [G * TPG * 128, CPT * SG], mybir.dt.bfloat16, kind="ExternalInput"
    ).ap()
    vt = nc.dram_tensor("vt", [128, NCH * R], mybir.dt.bfloat16, kind="ExternalInput").ap()
    wt = nc.dram_tensor("wt", [R, D], mybir.dt.bfloat16, kind="ExternalInput").ap()
    rp = nc.dram_tensor("rp", [R, 128], mybir.dt.bfloat16, kind="ExternalInput").ap()
    out = nc.dram_tensor("out", [128, (SH // 128) * D], mybir.dt.int8,
                         kind="ExternalOutput").ap()

    with tile.TileContext(nc) as tc:
        with contextlib.ExitStack() as ctx:
            cpool = ctx.enter_context(tc.tile_pool(name="consts", bufs=1))
            xpool = ctx.enter_context(tc.tile_pool(name="x", bufs=6))
            vrpool = ctx.enter_context(tc.tile_pool(name="xvr", bufs=2))
            opool = ctx.enter_context(tc.tile_pool(name="ob", bufs=8))
            psA = ctx.enter_context(tc.tile_pool(name="psA", bufs=2, space="PSUM"))
            psB = ctx.enter_context(tc.tile_pool(name="psB", bufs=6, space="PSUM"))

            # consts head the ring
            vt_sb = cpool.tile([128, NCH * R], mybir.dt.bfloat16)
            nc.sync.dma_start(vt_sb[:], vt[:])
            wt_sb = cpool.tile([R, D], mybir.dt.bfloat16)
            nc.sync.dma_start(wt_sb[:], wt[:])
            rp_sb = cpool.tile([R, 128], mybir.dt.bfloat16)
            nc.sync.dma_start(rp_sb[:], rp[:])

            # the whole x stream as 512 KiB sub-DMAs
            xs = []
            for t in range(G * TPG):
                xt = xpool.tile([128, CPT * SG], mybir.dt.bfloat16, tag="xs")
                for q in range(NSUB):
                    cols = slice(q * CPS * SG, (q + 1) * CPS * SG)
                    nc.sync.dma_start(xt[:, cols], xtp[t * 128:(t + 1) * 128, cols])
                xs.append(xt)

            # wtr = wt[p%8]/16 via 8 repmat matmuls (also HAM warmup)
            wtr_sb = cpool.tile([128, D], mybir.dt.bfloat16)
            for j in range(D // 512):
                psw = psB.tile([128, 512], mybir.dt.float32, tag="ps")
                nc.tensor.matmul(psw[:], rp_sb[:], wt_sb[:, j * 512:(j + 1) * 512],
                                 start=True, stop=True)
                if j % 2 == 0:
                    nc.vector.tensor_copy(wtr_sb[:, j * 512:(j + 1) * 512], psw[:])
                else:
                    nc.scalar.copy(wtr_sb[:, j * 512:(j + 1) * 512], psw[:])
            # vtr: V replicated 16x along the free dim, 16 strided copies
            vtr_sb = cpool.tile([128, NCH * 128], mybir.dt.bfloat16)
            vtr_v = vtr_sb[:].rearrange("p (c t j) -> p c t j", t=rep, j=R)
            vt_v = vt_sb[:].rearrange("p (c j) -> p c j", j=R)
            for t in range(rep):
                if t % 2 == 0:
                    nc.vector.tensor_copy(vtr_v[:, :, t, :], vt_v)
                else:
                    nc.scalar.copy(vtr_v[:, :, t, :], vt_v)

            GROUPS = [(0, 512), (512, 512), (1024, 512), (1536, 256), (1792, 256)]
            for k, (rs, n) in enumerate(GROUPS):
                # ---- mm1: replicated xv^T [128, n] over 32 d-chunks ----
                ps_xv = psA.tile([128, n], mybir.dt.float32, tag="psxv", name="psxv")
                sl = rs % SG
                for ch in range(NCH):
                    t2, c = divmod(ch, CPT)
                    xt = xs[(rs // SG) * TPG + t2]
                    nc.tensor.matmul(
                        ps_xv[:],
                        vtr_sb[:, ch * 128:(ch + 1) * 128],
                        xt[:, c * SG + sl:c * SG + sl + n],
                        start=(ch == 0),
                        stop=(ch == NCH - 1),
                    )
                xvr = vrpool.tile([128, n], mybir.dt.bfloat16, tag="xvr", name="xvr")
                nc.vector.tensor_copy(xvr[:, :n // 2], ps_xv[:, :n // 2])
                nc.scalar.copy(xvr[:, n // 2:], ps_xv[:, n // 2:])

                # ---- mm2: out strips [128, 4096] int8 = xv @ W'^T ----
                for i in range(n // 128):
                    ob = opool.tile([128, D], mybir.dt.int8, tag="ob", name="ob")
                    for j in range(D // 512):
                        ps = psB.tile([128, 512], mybir.dt.float32, tag="ps", name="ps")
                        nc.tensor.matmul(
                            ps[:],
                            xvr[:, i * 128:(i + 1) * 128],
                            wtr_sb[:, j * 512:(j + 1) * 512],
                            start=True, stop=True,
                        )
                        dst = ob[:, j * 512:(j + 1) * 512]
                        if j % 2 == 0:
                            nc.vector.tensor_copy(dst, ps[:])
                        else:
                            nc.scalar.copy(dst, ps[:])
                    q = rs // 128 + i
                    nc.sync.dma_start(out[:, q * D:(q + 1) * D], ob[:])

    nc.compile()
    return nc


def _get_kernels():
    if "k" not in _cache:
        _cache["k"] = build_fused()
    return _cache["k"]


def _vt_layout(V, d, r):
    """[128, (d//128)*r] bf16 with vt[p, c*r + j] = V[j, c*128 + p]."""
    nch = d // 128
    return np.ascontiguousarray(
        V.reshape(r, nch, 128).transpose(2, 1, 0).reshape(128, nch * r)
    ).astype(BF16)


def _routing_weights(x, V_shared, U_shared, core_pool, core_keys):
    """Exact f64 routing on host -> per-batch (W'_b [R, D] bf16 scaled by
    1/s_d, s [B, D] f32 dequant scales)."""
    mean = x.mean(axis=1, dtype=np.float64)  # [B, D]
    centroid = 0.7 * x[:, -1, :].astype(np.float64) + 0.3 * mean
    c_n = centroid / np.maximum(
        np.linalg.norm(centroid, axis=-1, keepdims=True), EPS
    )
    kk = core_keys.astype(np.float64)
    k_n = kk / np.maximum(np.linalg.norm(kk, axis=-1, keepdims=True), EPS)
    sim = c_n @ k_n.T  # [B, K]
    logits = sim / TEMP
    e = np.exp(logits - logits.max(axis=-1, keepdims=True))
    w = e / e.sum(axis=-1, keepdims=True)
    Lam = np.einsum("bk,kij->bij", w, core_pool.astype(np.float64))  # [B, R, R]
    Wb = np.einsum("dr,brj->bjd", U_shared.astype(np.float64), Lam)  # [B, R, D]
    Vf = V_shared.astype(np.float64)
    C = Vf @ Vf.T  # [R, R]
    sig = np.sqrt(np.einsum("bjd,jk,bkd->bd", Wb, C, Wb))  # [B, D]
    s = (QC / 127.0) * np.maximum(sig, 1e-12)  # [B, D]
    wt_b = [np.ascontiguousarray(Wb[b] / s[b][None, :]).astype(BF16) for b in range(B)]
    return wt_b, s.astype(np.float32)


def _pack_xtp(xshard):
    """[SH, D] f32 -> [1024, 8192] bf16: tile t=g*2+t2 row p col c*SG+s
    = x[g*512 + s, (t2*16 + c)*128 + p]."""
    v = np.ascontiguousarray(
        xshard.reshape(G, SG, TPG, CPT, 128).transpose(0, 2, 4, 3, 1)
    )
    return v.reshape(G * TPG * 128, CPT * SG).astype(BF16)


def _rp_layout(r):
    """[r, 128] bf16, rp[k, m] = (m % r == k)/16: partition replicator."""
    m = np.arange(128)
    return ((m[None, :] % r == np.arange(r)[:, None]) / 16.0).astype(BF16)


def _shard_inputs(x, V_shared, U_shared, core_pool, core_keys):
    vt_np = _vt_layout(V_shared.astype(np.float32), D, R)
    rp_np = _rp_layout(R)
    wt_b, s = _routing_weights(x, V_shared, U_shared, core_pool, core_keys)
    in_maps = []
    for c in range(NCORES):
        b, h = c // 2, c % 2
        xtp_c = _pack_xtp(x[b, h * SH:(h + 1) * SH, :])
        in_maps.append({"xtp": xtp_c, "vt": vt_np, "wt": wt_b[b], "rp": rp_np})
    return in_maps, s


def kernel(x, V_shared, U_shared, core_pool, core_keys):
    x = np.asarray(x)
    V_shared = np.asarray(V_shared)
    U_shared = np.asarray(U_shared)
    core_pool = np.asarray(core_pool)
    core_keys = np.asarray(core_keys)

    nc = _get_kernels()
    core_ids = list(range(NCORES))
    in_maps, s = _shard_inputs(x, V_shared, U_shared, core_pool, core_keys)
    res = run_bass_kernel_spmd(nc, in_maps, core_ids).results

    out = np.empty((B, S, D), dtype=np.float32)
    for c in core_ids:
        b, h = c // 2, c % 2
        a = res[c]["out"].reshape(128, SH // 128, D).transpose(1, 0, 2)
        out[b, h * SH:(h + 1) * SH, :] = (
            a.reshape(SH, D).astype(np.float32) * s[b][None, :]
        )
    return out


# revision 19
# speedup vs baseline: 1.2902x; 1.0413x over previous
"""Trainium2 Bass kernel for nn_CASCADES_v8_ResonantCore (moe_routing):

Computation (per batch b):
    centroid = 0.7*x[b,-1,:] + 0.3*mean_s(x[b])
    w = softmax(cos_sim(centroid, core_keys)/TEMP)      # [K]
    Lam = sum_k w[k] * core_pool[k]                     # [R,R]
    out[b] = ((x[b] @ V^T) @ Lam^T) @ U^T               # [S,D]

Strategy (8 cores, data-parallel over (batch, seq-half)):
  - Host: exact f64 routing; W_b = (U @ Lam_b)^T folded to one [R, D]
    weight per batch.  Output is written int8 with a per-column scale
    s_d = 8*sigma_d/127 (sigma_d^2 = W_d^T (V V^T) W_d) folded into
    the weight; host dequantizes.  f32->int8 on DVE/ACT rounds-to-
    nearest and saturates (HW-probed), so |err| <= s_d/2 ~ 0.03 sigma.
  - Reads: 512 KiB sub-DMAs on the sync ring (consts first), so the
    first matmul starts ~11.5 us instead of ~18.6 us.  Writes: eager
    per-strip int8 on the scalar ring.
  - Device per seq-group: V replicated 16x along the free dim in SBUF,
    so 32 accumulating matmuls produce the replicated xv^T [128, n]
    directly in PSUM; one copy to SBUF; then 8 expansion matmuls per
    128-row strip with [128,512] f32->int8 drains alternating DVE/ACT.
  - Groups are (512,512,512,256,256) rows: the smaller tail groups
    shrink the serial mm1->mm2->drain->write chain after the 16.8 MiB
    read stream ends (~358 GB/s sustained per core).
  - HBM traffic per core: 16.8 MiB read + 8.4 MiB write ~= 25.3 MiB.
"""

import sys

sys.path.insert(0, "/opt/trn_rl_repo")

import contextlib

import ml_dtypes
import numpy as np

import concourse.bass as bass  # noqa: F401  (registers bass types)
import concourse.tile as tile
from concourse import bacc, mybir
from concourse.bass_utils import run_bass_kernel_spmd

BF16 = ml_dtypes.bfloat16

B, S, D, R, K = 4, 4096, 4096, 8, 4
NCORES = 8
SH = S // 2     # 2048 seq rows per core
G = 4           # 512-row read-tile groups per core
SG = SH // G    # 512
NCH = D // 128  # 32 d-chunks
TPG = 2         # x tiles per 512-row group ([128, 8192] each)
CPT = NCH // TPG  # 16 d-chunks per x tile
NSUB = 4        # read sub-DMAs per tile (512 KiB each)
CPS = CPT // NSUB
EPS, TEMP = 1e-8, 0.05
QC = 8.0        # int8 scale: s_d = QC * sigma_d / 127

_cache = {}


def build_fused():
    """xtp [1024, 8192] bf16, vt [128, 256] bf16, wt [8, 4096] bf16
    (per-column-scaled W'), rp [8, 128] bf16 -> out [128, 65536] int8
    with out[p, q*4096 + d] = out_rows[q*128 + p, d], q = row-block."""
    rep = 128 // R
    nc = bacc.Bacc("TRN2", target_bir_lowering=False, debug=False)
    xtp = nc.dram_tensor(
        "xtp", [G * TPG * 128, CPT * SG], mybir.dt.bfloat16, kind="ExternalInput"
    ).ap()
    vt = nc.dram_tensor("vt", [128, NCH * R], mybir.dt.bfloat16, kind="ExternalInput").ap()
    wt = nc.dram_tensor("wt", [R, D], mybir.dt.bfloat16, kind="ExternalInput").ap()
    rp = nc.dram_tensor("rp", [R, 128], mybir.dt.bfloat16, kind="ExternalInput").ap()
    out = nc.dram_tensor("out", [128, (SH // 128) * D], mybir.dt.int8,
                         kind="ExternalOutput").ap()

    with tile.TileContext(nc) as tc:
        with contextlib.ExitStack() as ctx:
            cpool = ctx.enter_context(tc.tile_pool(name="consts", bufs=1))
            xpool = ctx.enter_context(tc.tile_pool(name="x", bufs=6))
            vrpool = ctx.enter_context(tc.tile_pool(name="xvr", bufs=2))
            opool = ctx.enter_context(tc.tile_pool(name="ob", bufs=8))
            psA = ctx.enter_context(tc.tile_pool(name="psA", bufs=2, space="PSUM"))
            psB = ctx.enter_context(tc.tile_pool(name="psB", bufs=6, space="PSUM"))

            # consts head the ring
            vt_sb = cpool.tile([128, NCH * R], mybir.dt.bfloat16)
            nc.sync.dma_start(vt_sb[:], vt[:])
            wt_sb = cpool.tile([R, D], mybir.dt.bfloat16)
            nc.sync.dma_start(wt_sb[:], wt[:])
            rp_sb = cpool.tile([R, 128], mybir.dt.bfloat16)
            nc.sync.dma_start(rp_sb[:], rp[:])

            # the whole x stream as 512 KiB sub-DMAs
            xs = []
            for t in range(G * TPG):
                xt = xpool.tile([128, CPT * SG], mybir.dt.bfloat16, tag="xs")
                for q in range(NSUB):
                    cols = slice(q * CPS * SG, (q + 1) * CPS * SG)
                    nc.sync.dma_start(xt[:, cols], xtp[t * 128:(t + 1) * 128, cols])
                xs.append(xt)

            # wtr = wt[p%8]/16 via 8 repmat matmuls (also HAM warmup)
            wtr_sb = cpool.tile([128, D], mybir.dt.bfloat16)
            for j in range(D // 512):
                psw = psB.tile([128, 512], mybir.dt.float32, tag="ps")
                nc.tensor.matmul(psw[:], rp_sb[:], wt_sb[:, j * 512:(j + 1) * 512],
                                 start=True, stop=True)
                if j % 2 == 0:
                    nc.vector.tensor_copy(wtr_sb[:, j * 512:(j + 1) * 512], psw[:])
                else:
                    nc.scalar.copy(wtr_sb[:, j * 512:(j + 1) * 512], psw[:])
            # vtr: V replicated 16x along the free dim, 16 strided copies
            vtr_sb = cpool.tile([128, NCH * 128], mybir.dt.bfloat16)
            vtr_v = vtr_sb[:].rearrange("p (c t j) -> p c t j", t=rep, j=R)
            vt_v = vt_sb[:].rearrange("p (c j) -> p c j", j=R)
            for t in range(rep):
                if t % 2 == 0:
                    nc.vector.tensor_copy(vtr_v[:, :, t, :], vt_v)
                else:
                    nc.scalar.copy(vtr_v[:, :, t, :], vt_v)

            GN = 256
            for k in range(SH // GN):
                rs = k * GN
                # ---- mm1: replicated xv^T [128, 256] over 32 d-chunks ----
                ps_xv = psA.tile([128, GN], mybir.dt.float32, tag="psxv", name="psxv")
                xt = xs[k]
                for ch in range(NCH):
                    nc.tensor.matmul(
                        ps_xv[:],
                        vtr_sb[:, ch * 128:(ch + 1) * 128],
                        xt[:, ch * GN:(ch + 1) * GN],
                        start=(ch == 0),
                        stop=(ch == NCH - 1),
                    )
                xvr = vrpool.tile([128, GN], mybir.dt.bfloat16, tag="xvr", name="xvr")
                nc.vector.tensor_copy(xvr[:, :GN // 2], ps_xv[:, :GN // 2])
                nc.scalar.copy(xvr[:, GN // 2:], ps_xv[:, GN // 2:])

                # ---- mm2: out strips [128, 4096] int8 = xv @ W'^T ----
                for i in range(GN // 128):
                    ob = opool.tile([128, D], mybir.dt.int8, tag="ob", name="ob")
                    for j in range(D // 512):
                        ps = psB.tile([128, 512], mybir.dt.float32, tag="ps", name="ps")
                        nc.tensor.matmul(
                            ps[:],
                            xvr[:, i * 128:(i + 1) * 128],
                            wtr_sb[:, j * 512:(j + 1) * 512],
                            start=True, stop=True,
                        )
                        dst = ob[:, j * 512:(j + 1) * 512]
                        if j % 2 == 0:
                            nc.vector.tensor_copy(dst, ps[:])
                        else:
                            nc.scalar.copy(dst, ps[:])
                    q = rs // 128 + i
                    nc.sync.dma_start(out[:, q * D:(q + 1) * D], ob[:])

    nc.compile()
    return nc


def _get_kernels():
    if "k" not in _cache:
        _cache["k"] = build_fused()
    return _cache["k"]


def _vt_layout(V, d, r):
    """[128, (d//128)*r] bf16 with vt[p, c*r + j] = V[j, c*128 + p]."""
    nch = d // 128
    return np.ascontiguousarray(
        V.reshape(r, nch, 128).transpose(2, 1, 0).reshape(128, nch * r)
    ).astype(BF16)


def _routing_weights(x, V_shared, U_shared, core_pool, core_keys):
    """Exact f64 routing on host -> per-batch (W'_b [R, D] bf16 scaled by
    1/s_d, s [B, D] f32 dequant scales)."""
    mean = x.mean(axis=1, dtype=np.float64)  # [B, D]
    centroid = 0.7 * x[:, -1, :].astype(np.float64) + 0.3 * mean
    c_n = centroid / np.maximum(
        np.linalg.norm(centroid, axis=-1, keepdims=True), EPS
    )
    kk = core_keys.astype(np.float64)
    k_n = kk / np.maximum(np.linalg.norm(kk, axis=-1, keepdims=True), EPS)
    sim = c_n @ k_n.T  # [B, K]
    logits = sim / TEMP
    e = np.exp(logits - logits.max(axis=-1, keepdims=True))
    w = e / e.sum(axis=-1, keepdims=True)
    Lam = np.einsum("bk,kij->bij", w, core_pool.astype(np.float64))  # [B, R, R]
    Wb = np.einsum("dr,brj->bjd", U_shared.astype(np.float64), Lam)  # [B, R, D]
    Vf = V_shared.astype(np.float64)
    C = Vf @ Vf.T  # [R, R]
    sig = np.sqrt(np.einsum("bjd,jk,bkd->bd", Wb, C, Wb))  # [B, D]
    s = (QC / 127.0) * np.maximum(sig, 1e-12)  # [B, D]
    wt_b = [np.ascontiguousarray(Wb[b] / s[b][None, :]).astype(BF16) for b in range(B)]
    return wt_b, s.astype(np.float32)


def _pack_xtp(xshard):
    """[SH, D] f32 -> [1024, 8192] bf16: tile t row p col c*256+s
    = x[t*256 + s, c*128 + p] (one tile per 256-row group)."""
    v = np.ascontiguousarray(
        xshard.reshape(8, 256, NCH, 128).transpose(0, 3, 2, 1)
    )
    return v.reshape(8 * 128, NCH * 256).astype(BF16)


def _rp_layout(r):
    """[r, 128] bf16, rp[k, m] = (m % r == k)/16: partition replicator."""
    m = np.arange(128)
    return ((m[None, :] % r == np.arange(r)[:, None]) / 16.0).astype(BF16)


def _shard_inputs(x, V_shared, U_shared, core_pool, core_keys):
    vt_np = _vt_layout(V_shared.astype(np.float32), D, R)
    rp_np = _rp_layout(R)
    wt_b, s = _routing_weights(x, V_shared, U_shared, core_pool, core_keys)
    in_maps = []
    for c in range(NCORES):
        b, h = c // 2, c % 2
        xtp_c = _pack_xtp(x[b, h * SH:(h + 1) * SH, :])
        in_maps.append({"xtp": xtp_c, "vt": vt_np, "wt": wt_b[b], "rp": rp_np})
    return in_maps, s


def kernel(x, V_shared, U_shared, core_pool, core_keys):
    x = np.asarray(x)
    V_shared = np.asarray(V_shared)
    U_shared = np.asarray(U_shared)
    core_pool = np.asarray(core_pool)
    core_keys = np.asarray(core_keys)

    nc = _get_kernels()
    core_ids = list(range(NCORES))
    in_maps, s = _shard_inputs(x, V_shared, U_shared, core_pool, core_keys)
    res = run_bass_kernel_spmd(nc, in_maps, core_ids).results

    out = np.empty((B, S, D), dtype=np.float32)
    for c in core_ids:
        b, h = c // 2, c % 2
        a = res[c]["out"].reshape(128, SH // 128, D).transpose(1, 0, 2)
        out[b, h * SH:(h + 1) * SH, :] = (
            a.reshape(SH, D).astype(np.float32) * s[b][None, :]
        )
    return out
